# revision 44
# baseline (speedup 1.0000x reference)
"""Trainium2 Bass kernel for MinibatchDiscrimination.

Reference computation:
    M = (x @ T.reshape(2048, 4096)).reshape(256, 128, 32)       # "matrices"
    norm[i,j,f] = sum_k |M[i,f,k] - M[j,f,k]|
    o_b[j,f]    = sum_i exp(-norm[i,j,f])
    out         = concat([x, o_b], axis=1)                       # [256, 2176]

Sharding: the OUT_FEATURES dim (128) is split across the 8 cores (16 features
each). Each core then owns the full 256x256 pairwise problem for its features:
  - matmul slice:   M_c^T [512=(k,f), 256 j]  (1/8 of the full matmul, no
    duplicated work, no collectives; fk ordering is k-major so the k-reduce
    matmul can use one shared stationary matrix)
  - pairwise:       the ISA has no abs op for tensor_scalar, so use
                    sum_k |d_k| = 2*sum_k relu(d_k) - (S_j - S_i) with
                    S_j = sum_k M[j,f,k] precomputed. relu(d) is ONE fused
                    DVE op per (i, fk-tile): tensor_scalar(subtract, max, 0)
                    at bf16 (2x mode; the per-partition scalar AP blocks 4x).
                    ~1/8 of these run on ACT via activation(Relu, bias=-col_i)
                    reading a private copy of M^T to avoid SBUF contention.
  - k-reduce:       PE matmul with a 2.0-valued matrix B[p,f]=2*1[p%16==f],
                    PSUM-accumulated over the 4 fk tiles; the -S_j term is
                    added by one extra matmul vs a partition-replicated
                    (-S_j/16) tile using the SAME stationary B (no reloads).
  - exp+accum:      ACT activation(Exp, scale=-1, bias=-S_i, accum_out) gives
                    sum_j exp(-norm[i,j,:]) = o_b[i,:] (norm is symmetric)

Numerics: M has std ~45 so off-diagonal L1 norms are ~1600 and exp(-norm)
underflows to exactly 0.0f, same as the f32 reference; the only surviving
term is the diagonal, which is exactly 0 by construction: every path
(tensor operand, scalar operand, S_j matmul contribution, exp bias) uses
the same bf16 rounding or its exact f32 upcast, so the subtractions cancel
exactly at i==j and the kernel reproduces the reference bit-for-bit.
Measured: ~240 us on HW, rel err 0.0 (exact).
"""

import sys

if "/opt/trn_rl_repo" not in sys.path:
    sys.path.insert(0, "/opt/trn_rl_repo")

import ml_dtypes
import numpy as np

import concourse.bacc as bacc
import concourse.bass as bass
import concourse.mybir as mybir
import concourse.tile as tile
from concourse.bass_utils import run_bass_kernel_spmd

N = 256
IN_F = 2048
OUT_F = 128
KD = 32
NCORES = 8
F_LOC = OUT_F // NCORES        # 16 features per core
FK = F_LOC * KD                # 512
NT = FK // 128                 # 4 fk tiles of 128 partitions
NCT = IN_F // 128              # 16 contraction tiles

F32 = mybir.dt.float32
BF16 = mybir.dt.bfloat16

_CACHE = {}


def _build():
    nc = bacc.Bacc()
    xT_d = nc.dram_tensor("xT", [IN_F, N], F32, kind="ExternalInput")
    Tsl_d = nc.dram_tensor("Tsl", [IN_F, FK], F32, kind="ExternalInput")
    Bm_d = nc.dram_tensor("Bm", [128, F_LOC], BF16, kind="ExternalInput")
    ob_d = nc.dram_tensor("ob", [F_LOC, N], F32, kind="ExternalOutput")

    with tile.TileContext(nc) as tc:
        with (
            tc.tile_pool(name="persist", bufs=1) as pp,
            tc.tile_pool(name="stage", bufs=NCT) as sp,
            tc.tile_pool(name="ad", bufs=8) as adp,
            tc.tile_pool(name="es", bufs=4) as esp,
            tc.tile_pool(name="psum", bufs=2, space=bass.MemorySpace.PSUM) as psp,
            tc.tile_pool(name="npsum", bufs=6, space=bass.MemorySpace.PSUM) as npp,
        ):
            # ---- load constants & inputs, cast to bf16 ----
            Bsb = pp.tile([128, F_LOC], BF16, tag="Bsb")
            nc.sync.dma_start(Bsb[:], Bm_d[:])

            xb = []
            tb = []
            for ct in range(NCT):
                xs = sp.tile([128, N], F32, tag="xstage")
                nc.sync.dma_start(xs[:], xT_d[ct * 128:(ct + 1) * 128, :])
                xbt = pp.tile([128, N], BF16, tag=f"xb{ct}")
                nc.scalar.copy(xbt[:], xs[:])
                xb.append(xbt)

                ts_ = sp.tile([128, FK], F32, tag="tstage")
                nc.sync.dma_start(ts_[:], Tsl_d[ct * 128:(ct + 1) * 128, :])
                tbt = pp.tile([128, FK], BF16, tag=f"tb{ct}")
                nc.vector.tensor_copy(tbt[:], ts_[:])
                tb.append(tbt)

            # ---- phase 1: M^T tiles [128 fk', 256 j], fk' = k*16 + f ----
            Mt = []   # bf16
            Mf = []   # exact f32 upcast of the bf16 values
            Mt2 = []  # private bf16 copy for ACT relu ops
            Mn = []   # exact f32 negation of the bf16 values
            for t in range(NT):
                mp = psp.tile([128, N], F32, tag="mpsum")
                for ct in range(NCT):
                    nc.tensor.matmul(
                        mp[:],
                        tb[ct][:, t * 128:(t + 1) * 128],
                        xb[ct][:],
                        start=(ct == 0),
                        stop=(ct == NCT - 1),
                    )
                mt = pp.tile([128, N], BF16, tag=f"Mt{t}")
                nc.vector.tensor_copy(mt[:], mp[:])
                mf = pp.tile([128, N], F32, tag=f"Mf{t}")
                nc.vector.tensor_copy(mf[:], mt[:])
                # private copies for the ACT-side relu ops: separate SBUF
                # banks so ACT and DVE don't contend reading the same tile
                m2 = pp.tile([128, N], BF16, tag=f"Mt2_{t}")
                nc.scalar.copy(m2[:], mp[:])
                mn = pp.tile([128, N], F32, tag=f"Mn{t}")
                nc.vector.tensor_scalar(mn[:], mt[:], -1.0, None, mybir.AluOpType.mult)
                Mt.append(mt)
                Mf.append(mf)
                Mt2.append(m2)
                Mn.append(mn)

            # ---- phase 1.5: row sums S_j = sum_k M[j,f,k] ----
            # Bsb holds 2.0 at (p, p%16): sjp = 2*S_j.
            # Xq[16g+r, e*N+j] = -S_j[r]/16 for all 8 groups g: the phase-2
            # matmul Bsb^T @ Xq then contributes 16 * (-S_j/16) = -S_j to the
            # PSUM bank using the SAME stationary as the relu reduce (no
            # weight swap). /16 and *16 are exact in bf16/f32, and the exp
            # bias SjF2 = 16*upcast(SjB16) matches the matmul path exactly,
            # so the diagonal still cancels to exp(0)=1.
            sjp = npp.tile([F_LOC, N], F32, tag="npsum", name="sjp")
            for t in range(NT):
                nc.tensor.matmul(
                    sjp[:], Bsb[:], Mt[t][:], start=(t == 0), stop=(t == NT - 1)
                )
            SjB16 = pp.tile([F_LOC, 2 * N], BF16, tag="SjB16")
            nc.vector.tensor_scalar(
                SjB16[:, 0:N], sjp[:], -1.0 / 32.0, None, mybir.AluOpType.mult
            )
            nc.vector.tensor_copy(SjB16[:, N:2 * N], SjB16[:, 0:N])
            Xq = pp.tile([128, 2 * N], BF16, tag="Xq")
            for g in range(128 // F_LOC):
                nc.sync.dma_start(Xq[g * F_LOC:(g + 1) * F_LOC, :], SjB16[:])
            SjF2 = pp.tile([F_LOC, N], F32, tag="SjF2")
            nc.vector.tensor_scalar(
                SjF2[:], SjB16[:, 0:N], 16.0, None, mybir.AluOpType.mult
            )

            ob_sb = pp.tile([F_LOC, N], F32, tag="ob_sb")

            # ---- phase 2: relu(d) / reduce / exp-accum ----
            # norm[i,j,f] = sum_k |d_k| = 2*sum_k relu(d_k) - (S_j - S_i)
            # npm = DMA(-S_j) + sum_t (2B)^T R_t   (PSUM accumulation)
            # o_b[i,f] = sum_j exp(-npm[f,j] - S_i[f])   (norm symmetry)
            opidx = 0
            for q in range(N // 2):
                ads = [
                    adp.tile([128, 2 * N], BF16, tag=f"ad{t}", name=f"ad{t}")
                    for t in range(NT)
                ]
                for e in range(2):
                    i = 2 * q + e
                    for t in range(NT):
                        dst = ads[t][:, e * N:(e + 1) * N]
                        if opidx % 7 < 2:
                            nc.scalar.activation(
                                dst,
                                Mt2[t][:],
                                mybir.ActivationFunctionType.Relu,
                                bias=Mn[t][:, i:i + 1],
                                scale=1.0,
                            )
                        else:
                            nc.vector.tensor_scalar(
                                dst,
                                Mt[t][:],
                                Mf[t][:, i:i + 1],
                                0.0,
                                mybir.AluOpType.subtract,
                                mybir.AluOpType.max,
                            )
                        opidx += 1
                npm = npp.tile([F_LOC, 2 * N], F32, tag="npsum")
                nc.tensor.matmul(npm[:], Bsb[:], Xq[:], start=True, stop=False)
                for t in range(NT):
                    nc.tensor.matmul(
                        npm[:],
                        Bsb[:],
                        ads[t][:],
                        start=False,
                        stop=(t == NT - 1),
                    )
                for e in range(2):
                    i = 2 * q + e
                    es = esp.tile([F_LOC, N], BF16, tag="es")
                    nc.scalar.activation(
                        es[:],
                        npm[:, e * N:(e + 1) * N],
                        mybir.ActivationFunctionType.Exp,
                        scale=-1.0,
                        bias=SjF2[:, i:i + 1],
                    )
                    ej = esp.tile([F_LOC, N], BF16, tag="ej")
                    nc.vector.tensor_scalar(
                        ej[:],
                        es[:],
                        1.0,
                        None,
                        mybir.AluOpType.mult,
                        mybir.AluOpType.add,
                        accum_out=ob_sb[:, i:i + 1],
                    )

            nc.sync.dma_start(ob_d[:], ob_sb[:])

    nc.compile()
    return nc


def _strip_redundant_self_waits(nc):
    """Remove same-engine semaphore waits that are provably satisfied.

    Walrus codegen has a small fixed number of sync-wait slots per ISA
    instruction struct (1 for Activation/DMA, 2 for Matmult) and errors out
    with "Too many sync wait commands" when Tile emits more. Some of Tile's
    emitted waits are an instruction waiting on its *own* engine's semaphore
    for a count already reached earlier in that engine's (serial, in-order)
    instruction stream — always satisfied at issue time. Strip exactly
    those. DMA-completion semaphores are excluded: their increments fire at
    transfer completion, not in engine order.
    """
    def walk(blocks, out):
        for bb in blocks:
            for ins in bb.instructions:
                out.append(ins)
                inner = getattr(ins, "blocks", None)
                if inner:
                    walk(inner, out)

    flat = []
    for f in nc.m.functions:
        walk(f.blocks, flat)

    # semaphore -> set of (engine, is_dma) updaters
    updaters = {}
    for ins in flat:
        si = getattr(ins, "sync_info", None)
        if si is None:
            continue
        is_dma = isinstance(ins, mybir.InstDMACopy)
        for u in si.on_update:
            updaters.setdefault(u.ant_name, set()).add((ins.engine, is_dma))

    cum = {}
    n_stripped = 0
    for ins in flat:
        si = getattr(ins, "sync_info", None)
        if si is None:
            continue
        kept = []
        for w in si.on_wait:
            ups = updaters.get(w.ant_name, set())
            same_engine_compute = ups == {(ins.engine, False)} and not isinstance(
                ins, mybir.InstDMACopy
            )
            if (
                same_engine_compute
                and w.wait_value is not None
                and cum.get(w.ant_name, 0) >= w.wait_value
            ):
                n_stripped += 1
                continue
            kept.append(w)
        if len(kept) != len(si.on_wait):
            ins.sync_info = mybir.SyncInfo(on_wait=kept, on_update=list(si.on_update))
        for u in si.on_update:
            if u.update_value is not None:
                cum[u.ant_name] = cum.get(u.ant_name, 0) + u.update_value


def _get_nc():
    if "nc" not in _CACHE:
        _CACHE["nc"] = _build()
    return _CACHE["nc"]


def _prep_inputs(x, T):
    x = np.asarray(x, dtype=np.float32)
    T = np.asarray(T, dtype=np.float32)
    xT = np.ascontiguousarray(x.T)                      # [2048, 256]
    # 2.0-valued so the PE reduce computes 2*sum_k relu(d) directly
    Bm = 2.0 * np.tile(np.eye(F_LOC), (128 // F_LOC, 1))
    Bm = Bm.astype(ml_dtypes.bfloat16)
    in_maps = []
    for c in range(NCORES):
        f0 = c * F_LOC
        # k-major fk ordering: Tsl[c_, k*16+f] = T[c_, f0+f, k]
        Tsl = np.ascontiguousarray(
            T[:, f0:f0 + F_LOC, :].transpose(0, 2, 1).reshape(IN_F, FK)
        )
        in_maps.append({"xT": xT, "Tsl": Tsl, "Bm": Bm})
    return x, in_maps


def _run(x, T, trace=False):
    nc = _get_nc()
    x, in_maps = _prep_inputs(x, T)
    res = run_bass_kernel_spmd(nc, in_maps, core_ids=list(range(NCORES)), trace=trace)
    o_b = np.empty((N, OUT_F), dtype=np.float32)
    for c in range(NCORES):
        o_b[:, c * F_LOC:(c + 1) * F_LOC] = res.results[c]["ob"].T
    out = np.concatenate([x, o_b], axis=1)
    return out, res


def kernel(x, T):
    out, _ = _run(x, T, trace=False)
    return out


# revision 45
# speedup vs baseline: 1.3857x; 1.3857x over previous
"""Trainium2 Bass kernel for MinibatchDiscrimination.

Reference computation:
    M = (x @ T.reshape(2048, 4096)).reshape(256, 128, 32)       # "matrices"
    norm[i,j,f] = sum_k |M[i,f,k] - M[j,f,k]|
    o_b[j,f]    = sum_i exp(-norm[i,j,f])
    out         = concat([x, o_b], axis=1)                       # [256, 2176]

Sharding: the OUT_FEATURES dim (128) is split across the 8 cores (16 features
each). Each core then owns the full 256x256 pairwise problem for its features:
  - matmul slice:   M_c^T [512=(k,f), 256 j]  (1/8 of the full matmul, no
    duplicated work, no collectives; fk ordering is k-major so the k-reduce
    matmul can use one shared stationary matrix)
  - pairwise:       the ISA has no abs op for tensor_scalar, so use
                    sum_k |d_k| = 2*sum_k relu(d_k) - (S_j - S_i) with
                    S_j = sum_k M[j,f,k] precomputed. relu(d) is ONE fused
                    DVE op per (i, fk-tile): tensor_scalar(subtract, max, 0)
                    at bf16 (2x mode; the per-partition scalar AP blocks 4x).
                    ~1/8 of these run on ACT via activation(Relu, bias=-col_i)
                    reading a private copy of M^T to avoid SBUF contention.
  - k-reduce:       PE matmul with a 2.0-valued matrix B[p,f]=2*1[p%16==f],
                    PSUM-accumulated over the 4 fk tiles; the -S_j term is
                    added by one extra matmul vs a partition-replicated
                    (-S_j/16) tile using the SAME stationary B (no reloads).
  - exp+accum:      ACT activation(Exp, scale=-1, bias=-S_i, accum_out) gives
                    sum_j exp(-norm[i,j,:]) = o_b[i,:] (norm is symmetric)

Numerics: M has std ~45 so off-diagonal L1 norms are ~1600 and exp(-norm)
underflows to exactly 0.0f, same as the f32 reference; the only surviving
term is the diagonal, which is exactly 0 by construction: every path
(tensor operand, scalar operand, S_j matmul contribution, exp bias) uses
the same bf16 rounding or its exact f32 upcast, so the subtractions cancel
exactly at i==j and the kernel reproduces the reference bit-for-bit.
Measured: ~240 us on HW, rel err 0.0 (exact).
"""

import sys

if "/opt/trn_rl_repo" not in sys.path:
    sys.path.insert(0, "/opt/trn_rl_repo")

import ml_dtypes
import numpy as np

import concourse.bacc as bacc
import concourse.bass as bass
import concourse.mybir as mybir
import concourse.tile as tile
from concourse.bass_utils import run_bass_kernel_spmd

N = 256
IN_F = 2048
OUT_F = 128
KD = 32
NCORES = 8
F_LOC = OUT_F // NCORES        # 16 features per core
FK = F_LOC * KD                # 512
NT = FK // 128                 # 4 fk tiles of 128 partitions
NCT = IN_F // 128              # 16 contraction tiles

F32 = mybir.dt.float32
BF16 = mybir.dt.bfloat16

_CACHE = {}


def _build():
    nc = bacc.Bacc()
    xT_d = nc.dram_tensor("xT", [IN_F, N], F32, kind="ExternalInput")
    Tsl_d = nc.dram_tensor("Tsl", [IN_F, FK], F32, kind="ExternalInput")
    Bm_d = nc.dram_tensor("Bm", [128, F_LOC], BF16, kind="ExternalInput")
    ob_d = nc.dram_tensor("ob", [F_LOC, N], F32, kind="ExternalOutput")

    with tile.TileContext(nc) as tc:
        with (
            tc.tile_pool(name="persist", bufs=1) as pp,
            tc.tile_pool(name="stage", bufs=NCT) as sp,
            tc.tile_pool(name="ad", bufs=8) as adp,
            tc.tile_pool(name="es", bufs=4) as esp,
            tc.tile_pool(name="psum", bufs=2, space=bass.MemorySpace.PSUM) as psp,
            tc.tile_pool(name="npsum", bufs=6, space=bass.MemorySpace.PSUM) as npp,
        ):
            # ---- load constants & inputs, cast to bf16 ----
            Bsb = pp.tile([128, F_LOC], BF16, tag="Bsb")
            nc.sync.dma_start(Bsb[:], Bm_d[:])

            xb = []
            tb = []
            for ct in range(NCT):
                xs = sp.tile([128, N], F32, tag="xstage")
                nc.sync.dma_start(xs[:], xT_d[ct * 128:(ct + 1) * 128, :])
                xbt = pp.tile([128, N], BF16, tag=f"xb{ct}")
                nc.scalar.copy(xbt[:], xs[:])
                xb.append(xbt)

                ts_ = sp.tile([128, FK], F32, tag="tstage")
                nc.sync.dma_start(ts_[:], Tsl_d[ct * 128:(ct + 1) * 128, :])
                tbt = pp.tile([128, FK], BF16, tag=f"tb{ct}")
                nc.vector.tensor_copy(tbt[:], ts_[:])
                tb.append(tbt)

            # ---- phase 1: M^T tiles [128 fk', 256 j], fk' = k*16 + f ----
            Mt = []   # bf16
            Mf = []   # exact f32 upcast of the bf16 values
            Mt2 = []  # private bf16 copy for ACT relu ops
            Mn = []   # exact f32 negation of the bf16 values
            for t in range(NT):
                mp = psp.tile([128, N], F32, tag="mpsum")
                for ct in range(NCT):
                    nc.tensor.matmul(
                        mp[:],
                        tb[ct][:, t * 128:(t + 1) * 128],
                        xb[ct][:],
                        start=(ct == 0),
                        stop=(ct == NCT - 1),
                    )
                mt = pp.tile([128, N], BF16, tag=f"Mt{t}")
                nc.vector.tensor_copy(mt[:], mp[:])
                mf = pp.tile([128, N], F32, tag=f"Mf{t}")
                nc.vector.tensor_copy(mf[:], mt[:])
                # private copies for the ACT-side relu ops: separate SBUF
                # banks so ACT and DVE don't contend reading the same tile
                m2 = pp.tile([128, N], BF16, tag=f"Mt2_{t}")
                nc.scalar.copy(m2[:], mp[:])
                mn = pp.tile([128, N], F32, tag=f"Mn{t}")
                nc.vector.tensor_scalar(mn[:], mt[:], -1.0, None, mybir.AluOpType.mult)
                Mt.append(mt)
                Mf.append(mf)
                Mt2.append(m2)
                Mn.append(mn)

            # ---- phase 1.5: row sums S_j = sum_k M[j,f,k] ----
            # Bsb holds 2.0 at (p, p%16): sjp = 2*S_j.
            # Xq[16g+r, e*N+j] = -S_j[r]/16 for all 8 groups g: the phase-2
            # matmul Bsb^T @ Xq then contributes 16 * (-S_j/16) = -S_j to the
            # PSUM bank using the SAME stationary as the relu reduce (no
            # weight swap). /16 and *16 are exact in bf16/f32, and the exp
            # bias SjF2 = 16*upcast(SjB16) matches the matmul path exactly,
            # so the diagonal still cancels to exp(0)=1.
            sjp = npp.tile([F_LOC, N], F32, tag="npsum", name="sjp")
            for t in range(NT):
                nc.tensor.matmul(
                    sjp[:], Bsb[:], Mt[t][:], start=(t == 0), stop=(t == NT - 1)
                )
            SjB16 = pp.tile([F_LOC, 2 * N], BF16, tag="SjB16")
            nc.vector.tensor_scalar(
                SjB16[:, 0:N], sjp[:], -1.0 / 32.0, None, mybir.AluOpType.mult
            )
            nc.vector.tensor_copy(SjB16[:, N:2 * N], SjB16[:, 0:N])
            Xq = pp.tile([128, 2 * N], BF16, tag="Xq")
            for g in range(128 // F_LOC):
                nc.sync.dma_start(Xq[g * F_LOC:(g + 1) * F_LOC, :], SjB16[:])
            SjF2 = pp.tile([F_LOC, N], F32, tag="SjF2")
            nc.vector.tensor_scalar(
                SjF2[:], SjB16[:, 0:N], 16.0, None, mybir.AluOpType.mult
            )

            ob_sb = pp.tile([F_LOC, N], F32, tag="ob_sb")

            # ---- phase 2: relu(d) / reduce / exp-accum ----
            # norm[i,j,f] = sum_k |d_k| = 2*sum_k relu(d_k) - (S_j - S_i)
            # npm = DMA(-S_j) + sum_t (2B)^T R_t   (PSUM accumulation)
            # o_b[i,f] = sum_j exp(-npm[f,j] - S_i[f])   (norm symmetry)
            opidx = 0
            for q in range(N // 2):
                ads = [
                    adp.tile([128, 2 * N], BF16, tag=f"ad{t}", name=f"ad{t}")
                    for t in range(NT)
                ]
                for e in range(2):
                    i = 2 * q + e
                    for t in range(NT):
                        dst = ads[t][:, e * N:(e + 1) * N]
                        if opidx % 8 == 7:
                            nc.scalar.activation(
                                dst,
                                Mt2[t][:],
                                mybir.ActivationFunctionType.Relu,
                                bias=Mn[t][:, i:i + 1],
                                scale=1.0,
                            )
                        else:
                            nc.vector.tensor_scalar(
                                dst,
                                Mt[t][:],
                                Mf[t][:, i:i + 1],
                                0.0,
                                mybir.AluOpType.subtract,
                                mybir.AluOpType.max,
                            )
                        opidx += 1
                npm = npp.tile([F_LOC, 2 * N], F32, tag="npsum")
                nc.tensor.matmul(npm[:], Bsb[:], Xq[:], start=True, stop=False)
                for t in range(NT):
                    nc.tensor.matmul(
                        npm[:],
                        Bsb[:],
                        ads[t][:],
                        start=False,
                        stop=(t == NT - 1),
                    )
                for e in range(2):
                    i = 2 * q + e
                    es = esp.tile([F_LOC, N], BF16, tag="es")
                    nc.scalar.activation(
                        es[:],
                        npm[:, e * N:(e + 1) * N],
                        mybir.ActivationFunctionType.Exp,
                        scale=-1.0,
                        bias=SjF2[:, i:i + 1],
                        accum_out=ob_sb[:, i:i + 1],
                    )

            nc.sync.dma_start(ob_d[:], ob_sb[:])

    nc.compile()
    return nc


def _strip_redundant_self_waits(nc):
    """Remove same-engine semaphore waits that are provably satisfied.

    Walrus codegen has a small fixed number of sync-wait slots per ISA
    instruction struct (1 for Activation/DMA, 2 for Matmult) and errors out
    with "Too many sync wait commands" when Tile emits more. Some of Tile's
    emitted waits are an instruction waiting on its *own* engine's semaphore
    for a count already reached earlier in that engine's (serial, in-order)
    instruction stream — always satisfied at issue time. Strip exactly
    those. DMA-completion semaphores are excluded: their increments fire at
    transfer completion, not in engine order.
    """
    def walk(blocks, out):
        for bb in blocks:
            for ins in bb.instructions:
                out.append(ins)
                inner = getattr(ins, "blocks", None)
                if inner:
                    walk(inner, out)

    flat = []
    for f in nc.m.functions:
        walk(f.blocks, flat)

    # semaphore -> set of (engine, is_dma) updaters
    updaters = {}
    for ins in flat:
        si = getattr(ins, "sync_info", None)
        if si is None:
            continue
        is_dma = isinstance(ins, mybir.InstDMACopy)
        for u in si.on_update:
            updaters.setdefault(u.ant_name, set()).add((ins.engine, is_dma))

    cum = {}
    n_stripped = 0
    for ins in flat:
        si = getattr(ins, "sync_info", None)
        if si is None:
            continue
        kept = []
        for w in si.on_wait:
            ups = updaters.get(w.ant_name, set())
            same_engine_compute = ups == {(ins.engine, False)} and not isinstance(
                ins, mybir.InstDMACopy
            )
            if (
                same_engine_compute
                and w.wait_value is not None
                and cum.get(w.ant_name, 0) >= w.wait_value
            ):
                n_stripped += 1
                continue
            kept.append(w)
        if len(kept) != len(si.on_wait):
            ins.sync_info = mybir.SyncInfo(on_wait=kept, on_update=list(si.on_update))
        for u in si.on_update:
            if u.update_value is not None:
                cum[u.ant_name] = cum.get(u.ant_name, 0) + u.update_value


def _get_nc():
    if "nc" not in _CACHE:
        _CACHE["nc"] = _build()
    return _CACHE["nc"]


def _prep_inputs(x, T):
    x = np.asarray(x, dtype=np.float32)
    T = np.asarray(T, dtype=np.float32)
    xT = np.ascontiguousarray(x.T)                      # [2048, 256]
    # 2.0-valued so the PE reduce computes 2*sum_k relu(d) directly
    Bm = 2.0 * np.tile(np.eye(F_LOC), (128 // F_LOC, 1))
    Bm = Bm.astype(ml_dtypes.bfloat16)
    in_maps = []
    for c in range(NCORES):
        f0 = c * F_LOC
        # k-major fk ordering: Tsl[c_, k*16+f] = T[c_, f0+f, k]
        Tsl = np.ascontiguousarray(
            T[:, f0:f0 + F_LOC, :].transpose(0, 2, 1).reshape(IN_F, FK)
        )
        in_maps.append({"xT": xT, "Tsl": Tsl, "Bm": Bm})
    return x, in_maps


def _run(x, T, trace=False):
    nc = _get_nc()
    x, in_maps = _prep_inputs(x, T)
    res = run_bass_kernel_spmd(nc, in_maps, core_ids=list(range(NCORES)), trace=trace)
    o_b = np.empty((N, OUT_F), dtype=np.float32)
    for c in range(NCORES):
        o_b[:, c * F_LOC:(c + 1) * F_LOC] = res.results[c]["ob"].T
    out = np.concatenate([x, o_b], axis=1)
    return out, res


def kernel(x, T):
    out, _ = _run(x, T, trace=False)
    return out
